# revision 72
# baseline (speedup 1.0000x reference)
"""HSIC loss kernel for Trainium2 (8 NeuronCores, Bass/Tile).

hsic = sum(L * HKH) / (m-1)^2
     = (S_LK - (2/m) kv.lv + sK*sL/m^2) / (m-1)^2
where K = exp(-dx), L = exp(-dy) (Gaussian kernels, sigma=1),
kv/lv = row sums of K/L, sK/sL = total sums, S_LK = sum(K*L).

Sharding (symmetry-aware): the m x m pair space is tiled into 512x512
blocks; only the upper triangle (136 blocks = 8 cores x 17 slots, each core
getting 2 diagonal + 15 off-diagonal blocks) is computed. An off-diagonal
block (I,J) contributes its row sums to rows I (free-axis accumulation on
the Scalar engine) and, via K's symmetry, its column sums to rows J
(selector-column matmul on the Tensor engine). Its K*L sum counts twice.
Host combines the tiny per-core partial sums in float64.

On-chip per 128x512 tile:
  PE  : G_K = x_i.x_j - sqx_j/2   (bf16 matmul + rank-2 hi/lo -sqx/2 rows)
        G_L = y_i.y_j - sqy_j/2   (single K=18 augmented matmul)
  ACT : K = exp(2*G_K + bias_i),  bias_i = -sqx_i  (per-partition, fp32),
        accum_out giving the row sums for free. Same for L.
  DVE : P = K*L with accum_out giving sum(K*L) partials.
  PE  : column sums of K and L accumulate into one [4, 512] PSUM tile
        (partition r selected by a one-hot-column stationary), drained by
        a single DVE copy + DMA per segment.

All matmuls are bf16; exactness of the Gram diagonal (the only entries that
matter at fp32 scale) is preserved by computing the squared norms on host
from the *bf16-rounded* inputs and carrying -sq/2 as a hi/lo bf16 pair.

_build_program(reps) wraps the whole body in a hardware For_i loop; the
test harness times reps=R vs reps=1 to get the marginal per-iteration
device time, independent of host dispatch overhead.
"""

import numpy as np
import ml_dtypes

M = 8192
DX = 128
DY = 16
NCORES = 8
B = 512                  # block edge
NBLK = M // B            # 16 blocks per edge
NSLOT = 17               # blocks per core: 136 = 8*17
TPB = B // 128           # i-tiles per block = 4
NACC = NSLOT * TPB       # accumulator columns = 68
W = NSLOT * B            # gathered free width = 8704

# Segments: (base slot, n half-blocks, is_diag)
SEGMENTS = (
    [(0, 1, True), (1, 1, True)]
    + [(2 + 2 * p, 2, False) for p in range(7)]
    + [(16, 1, False)]
)
NSEG = len(SEGMENTS)     # 10

_CACHE = {}

# Concurrency switches: place the aug / y matmuls on their own PE row
# groups (rows 64-65 / 96-113) so they overlap the fp8-DoubleRow x-matmul
# (rows 0-63) on hardware. Serial placement (base 0) forces FIFO order.
AUG_SERIAL = False
Y_SERIAL = False


def _core_slots():
    """Per-core block lists: [(I,J), ...] len 17.

    Positional layout: slots 0-1 diagonal singles; slots (2,3),(4,5)...(14,15)
    are same-I pairs (processed as one 1024-wide tile); slot 16 a single.
    120 off-diag blocks = 56 same-I pairs + 8 singles = 8 cores x (7 pairs + 1).
    """
    diag = [(d, d) for d in range(NBLK)]
    pairs, singles = [], []
    for i in range(NBLK):
        row = [(i, j) for j in range(i + 1, NBLK)]
        while len(row) >= 2:
            pairs.append((row.pop(0), row.pop(0)))
        if row:
            singles.append(row[0])
    assert len(pairs) == 7 * NCORES and len(singles) == NCORES
    slots = []
    for c in range(NCORES):
        sl = [diag[2 * c], diag[2 * c + 1]]
        for a, b in pairs[c::NCORES]:
            sl += [a, b]
        sl.append(singles[c])
        slots.append(sl)
    return slots


def _build_program(reps=1, mode="full"):
    """Build + compile the SPMD Bass program (identical for all cores).

    mode: "full" = normal; "noload" = input DMAs hoisted out of the reps
    loop (probe: is the loop DMA-bound?); "dmaonly" = loads but no compute
    (probe: pure DMA+sync cost). Probe modes are for timing bisection only.
    """
    from contextlib import ExitStack

    import concourse.bacc as bacc
    import concourse.tile as tile
    from concourse import mybir

    nc = bacc.Bacc(
        "TRN2",
        target_bir_lowering=False,
        debug=False,
        num_devices=NCORES,
    )
    bf16 = mybir.dt.bfloat16
    f32 = mybir.dt.float32
    f8 = mybir.dt.float8e4

    # Per-core DRAM inputs (host gathers the per-slot I/J column blocks).
    # The narrow tensors (xsq/ylhs/yrhs, few partitions each) load per-slot
    # from the Pool SWDGE queue whose triggers cost ~25ns, keeping the
    # in-order SP queue (500ns/trigger) free for the wide lhsx/rhsx stream
    # and away from the per-segment colKL drains.
    # x rides in fp8 with DoubleRow pairing: [64 partitions, 2, W] where
    # (k2, j) <-> feature dim 2*k2+j. The fp8 rounding is absorbed exactly
    # by computing the squared norms from the rounded values on host.
    lhsx_d = nc.dram_tensor("lhsx", [DX // 2, 2, W], f8, kind="ExternalInput").ap()
    rhsx_d = nc.dram_tensor("rhsx", [DX // 2, 2, W], f8, kind="ExternalInput").ap()
    # The two diagonal slots use a bf16 Gram instead: the fp8 DoubleRow
    # accumulator on HW carries a ~1e-4-relative deficit, visible only where
    # the Gram diagonal must cancel the squared norms exactly. dxx holds the
    # diag slots' columns (I == J, so one tensor serves both operands).
    dxx_d = nc.dram_tensor("dxx", [DX, 2 * B], bf16, kind="ExternalInput").ap()
    xsq_d = nc.dram_tensor("xsq", [2, W], bf16, kind="ExternalInput").ap()
    ylhs_d = nc.dram_tensor("ylhs", [DY + 2, W], bf16, kind="ExternalInput").ap()
    yrhs_d = nc.dram_tensor("yrhs", [DY + 2, W], bf16, kind="ExternalInput").ap()
    bxy_d = nc.dram_tensor("bxy", [128, 2 * NACC], f32, kind="ExternalInput").ap()

    accK_d = nc.dram_tensor("accK", [128, NACC], f32, kind="ExternalOutput").ap()
    accL_d = nc.dram_tensor("accL", [128, NACC], f32, kind="ExternalOutput").ap()
    accP_d = nc.dram_tensor("accP", [128, NACC], f32, kind="ExternalOutput").ap()
    # Column sums, streamed per segment: rows 4s..4s+2*nh hold
    # [K h0, L h0, K h1, L h1] for segment s.
    colKL_d = nc.dram_tensor("colKL", [4 * NSEG, B], f32, kind="ExternalOutput").ap()

    with tile.TileContext(nc) as tc, ExitStack() as ctx:
        singles = ctx.enter_context(tc.tile_pool(name="singles", bufs=1))
        work = ctx.enter_context(tc.tile_pool(name="work", bufs=4))
        psum = ctx.enter_context(tc.tile_pool(name="psum", bufs=2, space="PSUM"))

        # Operands of a tile_position=(r, 0) matmul must live at SBUF base
        # partition r (walrus: in_base_partition == tile_pos). The aug pair
        # sits at partitions 64-65, the y operands at 96-113; allocation
        # cost is unchanged (SBUF reserves the byte range, not partitions).
        ab = 0 if AUG_SERIAL else 64
        yb = 0 if Y_SERIAL else 96
        lhsx = singles.tile([DX // 2, 2, W], f8)
        rhsx = singles.tile([DX // 2, 2, W], f8)
        dxx = singles.tile([DX, 2 * B], bf16)
        xsq_t = singles.tile([ab + 2, W], bf16)
        xsq = xsq_t[ab:, :]
        ylhs_t = singles.tile([yb + DY + 2, W], bf16)
        ylhs = ylhs_t[yb:, :]
        yrhs_t = singles.tile([yb + DY + 2, W], bf16)
        yrhs = yrhs_t[yb:, :]
        bxy = singles.tile([128, 2 * NACC], f32)
        bx = bxy[:, :NACC]
        by = bxy[:, NACC:]
        ones2_t = singles.tile([ab + 2, 128], bf16)
        ones2 = ones2_t[ab:, :]
        ones128 = singles.tile([128, 1], bf16)
        accK = singles.tile([128, NACC], f32)
        accL = singles.tile([128, NACC], f32)
        accP = singles.tile([128, NACC], f32)

        nc.vector.memset(ones2, 1.0)
        nc.vector.memset(ones128, 1.0)

        exp = mybir.ActivationFunctionType.Exp
        mult = mybir.AluOpType.mult

        # Dummy activation at t~0 pulls the exp table load off the
        # startup-critical path (it would otherwise run after the first
        # tile's data wait).
        warm = singles.tile([1, 8], f32)
        nc.vector.memset(warm, 0.0)
        nc.scalar.activation(
            out=warm, in_=warm, func=exp, bias=0.0, scale=1.0
        )

        def loads():
            # Slot 0 loads first (compute-critical), then the bulk prefetch,
            # all on SP's HWDGE queue. SP only carries input prefetch, which
            # is self-paced; the per-segment output drains go via Pool.
            j0 = slice(0, B)
            nc.sync.dma_start(out=bxy, in_=bxy_d)
            nc.sync.dma_start(out=xsq[:, j0], in_=xsq_d[:, j0])
            nc.sync.dma_start(out=dxx[:, :B], in_=dxx_d[:, :B])
            nc.sync.dma_start(out=ylhs[:, j0], in_=ylhs_d[:, j0])
            nc.sync.dma_start(out=yrhs[:, j0], in_=yrhs_d[:, j0])
            nc.sync.dma_start(out=dxx[:, B:], in_=dxx_d[:, B:])
            nc.sync.dma_start(out=rhsx[:, :, j0], in_=rhsx_d[:, :, j0])
            nc.sync.dma_start(out=lhsx[:, :, j0], in_=lhsx_d[:, :, j0])
            for s in range(1, NSLOT):
                js = slice(s * B, (s + 1) * B)
                nc.sync.dma_start(out=xsq[:, js], in_=xsq_d[:, js])
                nc.sync.dma_start(out=rhsx[:, :, js], in_=rhsx_d[:, :, js])
                nc.sync.dma_start(out=lhsx[:, :, js], in_=lhsx_d[:, :, js])
                nc.sync.dma_start(out=ylhs[:, js], in_=ylhs_d[:, js])
                nc.sync.dma_start(out=yrhs[:, js], in_=yrhs_d[:, js])

        def body():
            if mode == "empty":
                nc.gpsimd.memset(accK, 0.0)
                nc.gpsimd.memset(accL, 0.0)
                nc.gpsimd.memset(accP, 0.0)
                nc.sync.dma_start(out=accK_d, in_=accK)
                nc.sync.dma_start(out=accL_d, in_=accL)
                nc.sync.dma_start(out=accP_d, in_=accP)
                return
            if mode != "noload":
                loads()
            if mode == "dmaonly":
                return
            nc.gpsimd.memset(accK, 0.0)
            nc.gpsimd.memset(accL, 0.0)
            nc.gpsimd.memset(accP, 0.0)

            pending = []
            seg_tiles = {}

            def emit_colsums(seg, nh):
                # Column sums, packed into one PSUM bank at partitions
                # 0/32 (half 0) and 64/96 (half 1) via tile_position
                # column tiling, accumulated over the segment's 4 tiles.
                cb = psum.tile([128, B], f32, tag="cb", bufs=2)
                for t, (ksb_t, lsb_t) in enumerate(seg_tiles.pop(seg)):
                    for h in range(nh):
                        hs = slice(h * B, (h + 1) * B)
                        for q, src in enumerate((ksb_t, lsb_t)):
                            pk = 64 * h + 32 * q
                            nc.tensor.matmul(
                                cb[pk : pk + 1, :],
                                ones128,
                                src[:, hs],
                                start=(t == 0),
                                stop=(t == TPB - 1),
                                tile_position=(0, pk),
                            )
                # Drain rows 0/32/64/96. Engine APs must start at a
                # 32-aligned partition, so the SBUF staging tile keeps the
                # same row positions and each row DMAs separately. Early
                # segments drain via Pool's SWDGE queue (keeps SP's
                # in-order queue free for input prefetch); the tail
                # segments use SP's lower-latency HWDGE path.
                nrow = 2 * nh
                ckl = work.tile([97, B], f32, tag="ckl", bufs=2)
                for r in range(nrow):
                    nc.vector.tensor_copy(
                        out=ckl[32 * r : 32 * r + 1],
                        in_=cb[32 * r : 32 * r + 1, :],
                    )
                eng = nc.sync if seg >= NSEG - 2 else nc.gpsimd
                for r in range(nrow):
                    eng.dma_start(
                        out=colKL_d[4 * seg + r : 4 * seg + r + 1, :],
                        in_=ckl[32 * r : 32 * r + 1],
                    )

            for seg, (s0, nh, diag) in enumerate(SEGMENTS):
                jw = nh * B
                kls = seg_tiles.setdefault(seg, [])
                for t in range(TPB):
                    col = s0 * TPB + t
                    isl = slice(s0 * B + t * 128, s0 * B + (t + 1) * 128)
                    # With column sums deferred a full segment, the binding
                    # PSUM WAR wait is gk's (aug of t+1 behind exp-K of t),
                    # so gk gets the double buffer.
                    gk = psum.tile([128, 2 * B], f32, tag="gk", bufs=2)
                    gl = psum.tile([128, 2 * B], f32, tag="gl", bufs=1)
                    for h in range(nh):
                        js = slice((s0 + h) * B, (s0 + h + 1) * B)
                        hs = slice(h * B, (h + 1) * B)
                        # The fp8 DoubleRow x-matmul occupies only array
                        # rows 0-63, so the aug (rows 64-65) and y (rows
                        # 96-113) matmuls run concurrently with it on HW.
                        # aug is issued first with start=True; its 2-deep
                        # pipeline drains each column ~60 cycles before the
                        # x-matmul's 64-deep one, so the clear always lands
                        # before the accumulate.
                        if mode == "nox":
                            nc.tensor.matmul(
                                gk[:, hs], ones2, xsq[:, js],
                                start=True, stop=True, tile_position=(ab, 0),
                            )
                        elif diag:
                            # bf16 Gram for the diagonal slots (exact
                            # cancellation of the squared norms).
                            nc.tensor.matmul(
                                gk[:, hs], ones2, xsq[:, js],
                                start=True, stop=False, tile_position=(ab, 0),
                            )
                            nc.tensor.matmul(
                                gk[:, hs],
                                dxx[:, s0 * B + t * 128 : s0 * B + (t + 1) * 128],
                                dxx[:, s0 * B : (s0 + 1) * B],
                                start=False,
                                stop=True,
                            )
                        else:
                            nc.tensor.matmul(
                                gk[:, hs], ones2, xsq[:, js],
                                start=True, stop=False, tile_position=(ab, 0),
                            )
                            nc.tensor.matmul(
                                gk[:, hs],
                                lhsx[:, :, isl],
                                rhsx[:, :, js],
                                start=False,
                                stop=True,
                                perf_mode=mybir.MatmulPerfMode.DoubleRow,
                            )
                        nc.tensor.matmul(
                            gl[:, hs], ylhs[:, isl], yrhs[:, js],
                            start=True, stop=True, tile_position=(yb, 0),
                        )
                    if mode == "peonly":
                        continue
                    # bufs=10: each segment's K/L tiles stay alive through
                    # the NEXT segment, so the deferred column-sum matmuls
                    # never make the in-order PE queue wait on ACT.
                    ksb = work.tile([128, 2 * B], bf16, tag="ksb", bufs=10)
                    lsb = work.tile([128, 2 * B], bf16, tag="lsb", bufs=10)
                    psb = None
                    if mode != "nodve":
                        psb = work.tile([128, 2 * B], bf16, tag="psb", bufs=6)
                    # Diagonal blocks are symmetric: row sums == colsums, so
                    # the ACT accumulator is skipped there.
                    nc.scalar.activation(
                        out=ksb[:, :jw],
                        in_=gk[:, :jw],
                        func=exp,
                        bias=bx[:, col : col + 1],
                        scale=2.0,
                        accum_out=None if diag else accK[:, col : col + 1],
                    )
                    nc.scalar.activation(
                        out=lsb[:, :jw],
                        in_=gl[:, :jw],
                        func=exp,
                        bias=by[:, col : col + 1],
                        scale=2.0,
                        accum_out=None if diag else accL[:, col : col + 1],
                    )
                    if mode != "nodve":
                        nc.vector.scalar_tensor_tensor(
                            out=psb[:, :jw],
                            in0=ksb[:, :jw],
                            scalar=1.0,
                            in1=lsb[:, :jw],
                            op0=mult,
                            op1=mult,
                            accum_out=accP[:, col : col + 1],
                        )
                    kls.append((ksb, lsb))
                # Column sums are deferred by one segment (emitted from the
                # pending list below) so their ACT-dependent semaphore waits
                # never head-of-line-block the next segment's compute
                # matmuls in the in-order PE queue.
                if mode not in ("nocolsum", "peonly"):
                    pending.append((seg, nh))
                    if len(pending) > 1:
                        emit_colsums(*pending.pop(0))
            while pending:
                emit_colsums(*pending.pop(0))

            nc.sync.dma_start(out=accK_d, in_=accK)
            nc.sync.dma_start(out=accL_d, in_=accL)
            nc.sync.dma_start(out=accP_d, in_=accP)

        if mode == "noload":
            loads()
        if reps > 1:
            with tc.For_i(0, reps):
                body()
        else:
            body()

    nc.compile()
    return nc


def _split_hi_lo(a):
    """Split float64 vector into hi+lo bf16 pair summing to ~a."""
    h = a.astype(ml_dtypes.bfloat16)
    l = (a - h.astype(np.float64)).astype(ml_dtypes.bfloat16)
    return h, l


def _prepare_in_maps(x, y):
    # Off-diagonal slots: x rounded to fp8e4m3 (TRN variant) for the
    # DoubleRow Gram matmul. Diagonal slots: bf16 Gram (dxx), because the
    # HW fp8 accumulator's tiny deficit breaks the exact diagonal
    # cancellation. Squared norms are computed from the rounded values of
    # whichever dtype that slot's Gram uses.
    xb8 = x.astype(ml_dtypes.float8_e4m3)
    xb16 = x.astype(ml_dtypes.bfloat16)
    yb = y.astype(ml_dtypes.bfloat16)
    sqx8 = (xb8.astype(np.float64) ** 2).sum(axis=1)  # [M]
    sqx16 = (xb16.astype(np.float64) ** 2).sum(axis=1)
    sqy = (yb.astype(np.float64) ** 2).sum(axis=1)

    xsqh8, xsql8 = _split_hi_lo(-0.5 * sqx8)
    xsqh16, xsql16 = _split_hi_lo(-0.5 * sqx16)
    ysqh, ysql = _split_hi_lo(-0.5 * sqy)

    xtb8 = np.ascontiguousarray(xb8.T)  # [DX, M]
    xtb16 = np.ascontiguousarray(xb16.T)
    ytb = np.ascontiguousarray(yb.T)  # [DY, M]
    xsq2_8 = np.stack([xsqh8, xsql8], axis=0)  # [2, M]
    xsq2_16 = np.stack([xsqh16, xsql16], axis=0)
    ysq2 = np.stack([ysqh, ysql], axis=0)
    ones_row = np.ones((2, M), dtype=ml_dtypes.bfloat16)
    ylhs_full = np.concatenate([ytb, ones_row], axis=0)  # [18, M]
    yrhs_full = np.concatenate([ytb, ysq2], axis=0)

    bslice = lambda a, blk: a[..., blk * B : (blk + 1) * B]

    in_maps = []
    for slots in _core_slots():
        is_diag = [I == J for I, J in slots]
        lhsx = np.concatenate([bslice(xtb8, I) for I, _ in slots], axis=1)
        rhsx = np.concatenate([bslice(xtb8, J) for _, J in slots], axis=1)
        # DoubleRow layout: [64, 2, W], (k2, j) <-> feature dim 2*k2+j
        lhsx = lhsx.reshape(DX // 2, 2, W)
        rhsx = rhsx.reshape(DX // 2, 2, W)
        dxx = np.concatenate(
            [bslice(xtb16, slots[s][0]) for s in range(2)], axis=1
        )
        xsq = np.concatenate(
            [
                bslice(xsq2_16 if dg else xsq2_8, J)
                for (_, J), dg in zip(slots, is_diag)
            ],
            axis=1,
        )
        ylhs = np.concatenate([bslice(ylhs_full, I) for I, _ in slots], axis=1)
        yrhs = np.concatenate([bslice(yrhs_full, J) for _, J in slots], axis=1)
        bxc = np.concatenate(
            [
                -(sqx16 if dg else sqx8)[I * B : (I + 1) * B].reshape(TPB, 128).T
                for (I, _), dg in zip(slots, is_diag)
            ],
            axis=1,
        ).astype(np.float32)
        byc = np.concatenate(
            [-sqy[I * B : (I + 1) * B].reshape(TPB, 128).T for I, _ in slots], axis=1
        ).astype(np.float32)
        in_maps.append(
            {
                "lhsx": np.ascontiguousarray(lhsx),
                "rhsx": np.ascontiguousarray(rhsx),
                "dxx": np.ascontiguousarray(dxx),
                "xsq": np.ascontiguousarray(xsq),
                "ylhs": np.ascontiguousarray(ylhs),
                "yrhs": np.ascontiguousarray(yrhs),
                "bxy": np.ascontiguousarray(np.concatenate([bxc, byc], axis=1)),
            }
        )
    return in_maps


def _combine(results):
    """Host-side reduction of per-core partial sums -> hsic scalar."""
    m = float(M)
    kv = np.zeros(M, dtype=np.float64)
    lv = np.zeros(M, dtype=np.float64)
    s_lk = 0.0
    for slots, res in zip(_core_slots(), results):
        aK = res["accK"].astype(np.float64)  # [128, NACC]
        aL = res["accL"].astype(np.float64)
        aP = res["accP"].astype(np.float64)
        cKL = res["colKL"].astype(np.float64)  # [4*NSEG, B]
        for seg, (s0, nh, diag) in enumerate(SEGMENTS):
            I = slots[s0][0]
            p_blk = aP[:, s0 * TPB : (s0 + 1) * TPB].sum()
            s_lk += p_blk if diag else 2.0 * p_blk
            if not diag:
                # row-side sums (cover all nh half-blocks' columns)
                for t in range(TPB):
                    rows = slice(I * B + t * 128, I * B + (t + 1) * 128)
                    kv[rows] += aK[:, s0 * TPB + t]
                    lv[rows] += aL[:, s0 * TPB + t]
            # col-side sums (for diag blocks these are also the row sums)
            for h in range(nh):
                J = slots[s0 + h][1]
                jrows = slice(J * B, (J + 1) * B)
                kv[jrows] += cKL[4 * seg + 2 * h]
                lv[jrows] += cKL[4 * seg + 2 * h + 1]
    sK = kv.sum()
    sL = lv.sum()
    hsic = (s_lk - (2.0 / m) * np.dot(kv, lv) + sK * sL / (m * m)) / (m - 1.0) ** 2
    return np.float32(hsic)


def get_program(reps=1, mode="full"):
    key = ("nc", reps, mode)
    if key not in _CACHE:
        _CACHE[key] = _build_program(reps, mode)
    return _CACHE[key]


def run_on_cores(in_maps):
    from concourse.bass_utils import run_bass_kernel_spmd

    nc = get_program()
    res = run_bass_kernel_spmd(nc, in_maps, core_ids=list(range(NCORES)))
    return res.results


def kernel(x, y):
    x = np.asarray(x)
    y = np.asarray(y)
    assert x.shape == (M, DX) and y.shape == (M, DY), (x.shape, y.shape)
    in_maps = _prepare_in_maps(x, y)
    results = run_on_cores(in_maps)
    return _combine(results)


# revision 75
# speedup vs baseline: 1.0535x; 1.0535x over previous
"""HSIC loss kernel for Trainium2 (8 NeuronCores, Bass/Tile).

hsic = sum(L * HKH) / (m-1)^2
     = (S_LK - (2/m) kv.lv + sK*sL/m^2) / (m-1)^2
where K = exp(-dx), L = exp(-dy) (Gaussian kernels, sigma=1),
kv/lv = row sums of K/L, sK/sL = total sums, S_LK = sum(K*L).

Sharding (symmetry-aware): the m x m pair space is tiled into 512x512
blocks; only the upper triangle (136 blocks = 8 cores x 17 slots, each core
getting 2 diagonal + 15 off-diagonal blocks) is computed. An off-diagonal
block (I,J) contributes its row sums to rows I (free-axis accumulation on
the Scalar engine) and, via K's symmetry, its column sums to rows J
(selector-column matmul on the Tensor engine). Its K*L sum counts twice.
Host combines the tiny per-core partial sums in float64.

On-chip per 128x512 tile:
  PE  : G_K = x_i.x_j - sqx_j/2   (bf16 matmul + rank-2 hi/lo -sqx/2 rows)
        G_L = y_i.y_j - sqy_j/2   (single K=18 augmented matmul)
  ACT : K = exp(2*G_K + bias_i),  bias_i = -sqx_i  (per-partition, fp32),
        accum_out giving the row sums for free. Same for L.
  DVE : P = K*L with accum_out giving sum(K*L) partials.
  PE  : column sums of K and L accumulate into one [4, 512] PSUM tile
        (partition r selected by a one-hot-column stationary), drained by
        a single DVE copy + DMA per segment.

All matmuls are bf16; exactness of the Gram diagonal (the only entries that
matter at fp32 scale) is preserved by computing the squared norms on host
from the *bf16-rounded* inputs and carrying -sq/2 as a hi/lo bf16 pair.

_build_program(reps) wraps the whole body in a hardware For_i loop; the
test harness times reps=R vs reps=1 to get the marginal per-iteration
device time, independent of host dispatch overhead.
"""

import numpy as np
import ml_dtypes

M = 8192
DX = 128
DY = 16
NCORES = 8
B = 512                  # block edge
NBLK = M // B            # 16 blocks per edge
NSLOT = 17               # blocks per core: 136 = 8*17
TPB = B // 128           # i-tiles per block = 4
NACC = NSLOT * TPB       # accumulator columns = 68
W = NSLOT * B            # gathered free width = 8704

# Segments: (base slot, n half-blocks, is_diag)
SEGMENTS = (
    [(0, 1, True), (1, 1, True)]
    + [(2 + 2 * p, 2, False) for p in range(7)]
    + [(16, 1, False)]
)
NSEG = len(SEGMENTS)     # 10

_CACHE = {}

# Concurrency switches: place the aug / y matmuls on their own PE row
# groups (rows 64-65 / 96-113) so they overlap the fp8-DoubleRow x-matmul
# (rows 0-63) on hardware. Serial placement (base 0) forces FIFO order.
AUG_SERIAL = False
Y_SERIAL = False
# gl double-buffered, gk single: measured faster once column sums are
# deferred (the PSUM budget only fits one double-buffered Gram tile).
GK_BUFS = 1


def _core_slots():
    """Per-core block lists: [(I,J), ...] len 17.

    Positional layout: slots 0-1 diagonal singles; slots (2,3),(4,5)...(14,15)
    are same-I pairs (processed as one 1024-wide tile); slot 16 a single.
    120 off-diag blocks = 56 same-I pairs + 8 singles = 8 cores x (7 pairs + 1).
    """
    diag = [(d, d) for d in range(NBLK)]
    pairs, singles = [], []
    for i in range(NBLK):
        row = [(i, j) for j in range(i + 1, NBLK)]
        while len(row) >= 2:
            pairs.append((row.pop(0), row.pop(0)))
        if row:
            singles.append(row[0])
    assert len(pairs) == 7 * NCORES and len(singles) == NCORES
    slots = []
    for c in range(NCORES):
        sl = [diag[2 * c], diag[2 * c + 1]]
        for a, b in pairs[c::NCORES]:
            sl += [a, b]
        sl.append(singles[c])
        slots.append(sl)
    return slots


def _build_program(reps=1, mode="full"):
    """Build + compile the SPMD Bass program (identical for all cores).

    mode: "full" = normal; "noload" = input DMAs hoisted out of the reps
    loop (probe: is the loop DMA-bound?); "dmaonly" = loads but no compute
    (probe: pure DMA+sync cost). Probe modes are for timing bisection only.
    """
    from contextlib import ExitStack

    import concourse.bacc as bacc
    import concourse.tile as tile
    from concourse import mybir

    nc = bacc.Bacc(
        "TRN2",
        target_bir_lowering=False,
        debug=False,
        num_devices=NCORES,
    )
    bf16 = mybir.dt.bfloat16
    f32 = mybir.dt.float32
    f8 = mybir.dt.float8e4

    # Per-core DRAM inputs (host gathers the per-slot I/J column blocks).
    # The narrow tensors (xsq/ylhs/yrhs, few partitions each) load per-slot
    # from the Pool SWDGE queue whose triggers cost ~25ns, keeping the
    # in-order SP queue (500ns/trigger) free for the wide lhsx/rhsx stream
    # and away from the per-segment colKL drains.
    # x rides in fp8 with DoubleRow pairing: [64 partitions, 2, W] where
    # (k2, j) <-> feature dim 2*k2+j. The fp8 rounding is absorbed exactly
    # by computing the squared norms from the rounded values on host.
    lhsx_d = nc.dram_tensor("lhsx", [DX // 2, 2, W], f8, kind="ExternalInput").ap()
    rhsx_d = nc.dram_tensor("rhsx", [DX // 2, 2, W], f8, kind="ExternalInput").ap()
    # The two diagonal slots use a bf16 Gram instead: the fp8 DoubleRow
    # accumulator on HW carries a ~1e-4-relative deficit, visible only where
    # the Gram diagonal must cancel the squared norms exactly. dxx holds the
    # diag slots' columns (I == J, so one tensor serves both operands).
    dxx_d = nc.dram_tensor("dxx", [DX, 2 * B], bf16, kind="ExternalInput").ap()
    xsq_d = nc.dram_tensor("xsq", [2, W], bf16, kind="ExternalInput").ap()
    ylhs_d = nc.dram_tensor("ylhs", [DY + 2, W], bf16, kind="ExternalInput").ap()
    yrhs_d = nc.dram_tensor("yrhs", [DY + 2, W], bf16, kind="ExternalInput").ap()
    bxy_d = nc.dram_tensor("bxy", [128, 2 * NACC], f32, kind="ExternalInput").ap()

    accK_d = nc.dram_tensor("accK", [128, NACC], f32, kind="ExternalOutput").ap()
    accL_d = nc.dram_tensor("accL", [128, NACC], f32, kind="ExternalOutput").ap()
    accP_d = nc.dram_tensor("accP", [128, NACC], f32, kind="ExternalOutput").ap()
    # Column sums, streamed per segment: rows 4s..4s+2*nh hold
    # [K h0, L h0, K h1, L h1] for segment s.
    colKL_d = nc.dram_tensor("colKL", [4 * NSEG, B], f32, kind="ExternalOutput").ap()

    with tile.TileContext(nc) as tc, ExitStack() as ctx:
        singles = ctx.enter_context(tc.tile_pool(name="singles", bufs=1))
        work = ctx.enter_context(tc.tile_pool(name="work", bufs=4))
        psum = ctx.enter_context(tc.tile_pool(name="psum", bufs=2, space="PSUM"))

        # Operands of a tile_position=(r, 0) matmul must live at SBUF base
        # partition r (walrus: in_base_partition == tile_pos). The aug pair
        # sits at partitions 64-65, the y operands at 96-113; allocation
        # cost is unchanged (SBUF reserves the byte range, not partitions).
        ab = 0 if AUG_SERIAL else 64
        yb = 0 if Y_SERIAL else 96
        lhsx = singles.tile([DX // 2, 2, W], f8)
        rhsx = singles.tile([DX // 2, 2, W], f8)
        dxx = singles.tile([DX, 2 * B], bf16)
        xsq_t = singles.tile([ab + 2, W], bf16)
        xsq = xsq_t[ab:, :]
        ylhs_t = singles.tile([yb + DY + 2, W], bf16)
        ylhs = ylhs_t[yb:, :]
        yrhs_t = singles.tile([yb + DY + 2, W], bf16)
        yrhs = yrhs_t[yb:, :]
        bxy = singles.tile([128, 2 * NACC], f32)
        bx = bxy[:, :NACC]
        by = bxy[:, NACC:]
        ones2_t = singles.tile([ab + 2, 128], bf16)
        ones2 = ones2_t[ab:, :]
        ones128 = singles.tile([128, 1], bf16)
        accK = singles.tile([128, NACC], f32)
        accL = singles.tile([128, NACC], f32)
        accP = singles.tile([128, NACC], f32)

        nc.vector.memset(ones2, 1.0)
        nc.vector.memset(ones128, 1.0)

        exp = mybir.ActivationFunctionType.Exp
        mult = mybir.AluOpType.mult

        # Dummy activation at t~0 pulls the exp table load off the
        # startup-critical path (it would otherwise run after the first
        # tile's data wait).
        warm = singles.tile([1, 8], f32)
        nc.vector.memset(warm, 0.0)
        nc.scalar.activation(
            out=warm, in_=warm, func=exp, bias=0.0, scale=1.0
        )

        def loads():
            # Slot 0 loads first (compute-critical), then the bulk prefetch,
            # all on SP's HWDGE queue. SP only carries input prefetch, which
            # is self-paced; the per-segment output drains go via Pool.
            j0 = slice(0, B)
            nc.sync.dma_start(out=bxy, in_=bxy_d)
            nc.sync.dma_start(out=xsq[:, j0], in_=xsq_d[:, j0])
            nc.sync.dma_start(out=dxx[:, :B], in_=dxx_d[:, :B])
            nc.sync.dma_start(out=ylhs[:, j0], in_=ylhs_d[:, j0])
            nc.sync.dma_start(out=yrhs[:, j0], in_=yrhs_d[:, j0])
            nc.sync.dma_start(out=dxx[:, B:], in_=dxx_d[:, B:])
            nc.sync.dma_start(out=rhsx[:, :, j0], in_=rhsx_d[:, :, j0])
            nc.sync.dma_start(out=lhsx[:, :, j0], in_=lhsx_d[:, :, j0])
            for s in range(1, NSLOT):
                js = slice(s * B, (s + 1) * B)
                nc.sync.dma_start(out=xsq[:, js], in_=xsq_d[:, js])
                nc.sync.dma_start(out=rhsx[:, :, js], in_=rhsx_d[:, :, js])
                nc.sync.dma_start(out=lhsx[:, :, js], in_=lhsx_d[:, :, js])
                nc.sync.dma_start(out=ylhs[:, js], in_=ylhs_d[:, js])
                nc.sync.dma_start(out=yrhs[:, js], in_=yrhs_d[:, js])

        def body():
            if mode == "empty":
                nc.gpsimd.memset(accK, 0.0)
                nc.gpsimd.memset(accL, 0.0)
                nc.gpsimd.memset(accP, 0.0)
                nc.sync.dma_start(out=accK_d, in_=accK)
                nc.sync.dma_start(out=accL_d, in_=accL)
                nc.sync.dma_start(out=accP_d, in_=accP)
                return
            if mode != "noload":
                loads()
            if mode == "dmaonly":
                return
            nc.gpsimd.memset(accK, 0.0)
            nc.gpsimd.memset(accL, 0.0)
            nc.gpsimd.memset(accP, 0.0)

            pending = []
            seg_tiles = {}

            def emit_colsums(seg, nh):
                # Column sums, packed into one PSUM bank at partitions
                # 0/32 (half 0) and 64/96 (half 1) via tile_position
                # column tiling, accumulated over the segment's 4 tiles.
                cb = psum.tile([128, B], f32, tag="cb", bufs=2)
                for t, (ksb_t, lsb_t) in enumerate(seg_tiles.pop(seg)):
                    for h in range(nh):
                        hs = slice(h * B, (h + 1) * B)
                        for q, src in enumerate((ksb_t, lsb_t)):
                            pk = 64 * h + 32 * q
                            nc.tensor.matmul(
                                cb[pk : pk + 1, :],
                                ones128,
                                src[:, hs],
                                start=(t == 0),
                                stop=(t == TPB - 1),
                                tile_position=(0, pk),
                            )
                # Drain rows 0/32/64/96. Engine APs must start at a
                # 32-aligned partition, so the SBUF staging tile keeps the
                # same row positions and each row DMAs separately. Early
                # segments drain via Pool's SWDGE queue (keeps SP's
                # in-order queue free for input prefetch); the tail
                # segments use SP's lower-latency HWDGE path.
                nrow = 2 * nh
                ckl = work.tile([97, B], f32, tag="ckl", bufs=2)
                for r in range(nrow):
                    nc.vector.tensor_copy(
                        out=ckl[32 * r : 32 * r + 1],
                        in_=cb[32 * r : 32 * r + 1, :],
                    )
                eng = nc.sync if seg >= NSEG - 2 else nc.gpsimd
                for r in range(nrow):
                    eng.dma_start(
                        out=colKL_d[4 * seg + r : 4 * seg + r + 1, :],
                        in_=ckl[32 * r : 32 * r + 1],
                    )

            for seg, (s0, nh, diag) in enumerate(SEGMENTS):
                jw = nh * B
                kls = seg_tiles.setdefault(seg, [])
                for t in range(TPB):
                    col = s0 * TPB + t
                    isl = slice(s0 * B + t * 128, s0 * B + (t + 1) * 128)
                    # With column sums deferred a full segment, one of gk/gl
                    # can double-buffer (PSUM fits 8 banks total with cb).
                    gk = psum.tile([128, 2 * B], f32, tag="gk", bufs=GK_BUFS)
                    gl = psum.tile([128, 2 * B], f32, tag="gl", bufs=3 - GK_BUFS)
                    for h in range(nh):
                        js = slice((s0 + h) * B, (s0 + h + 1) * B)
                        hs = slice(h * B, (h + 1) * B)
                        # The fp8 DoubleRow x-matmul occupies only array
                        # rows 0-63, so the aug (rows 64-65) and y (rows
                        # 96-113) matmuls run concurrently with it on HW.
                        # aug is issued first with start=True; its 2-deep
                        # pipeline drains each column ~60 cycles before the
                        # x-matmul's 64-deep one, so the clear always lands
                        # before the accumulate.
                        if mode == "nox":
                            nc.tensor.matmul(
                                gk[:, hs], ones2, xsq[:, js],
                                start=True, stop=True, tile_position=(ab, 0),
                            )
                        elif diag:
                            # bf16 Gram for the diagonal slots (exact
                            # cancellation of the squared norms).
                            nc.tensor.matmul(
                                gk[:, hs], ones2, xsq[:, js],
                                start=True, stop=False, tile_position=(ab, 0),
                            )
                            nc.tensor.matmul(
                                gk[:, hs],
                                dxx[:, s0 * B + t * 128 : s0 * B + (t + 1) * 128],
                                dxx[:, s0 * B : (s0 + 1) * B],
                                start=False,
                                stop=True,
                            )
                        else:
                            nc.tensor.matmul(
                                gk[:, hs], ones2, xsq[:, js],
                                start=True, stop=False, tile_position=(ab, 0),
                            )
                            nc.tensor.matmul(
                                gk[:, hs],
                                lhsx[:, :, isl],
                                rhsx[:, :, js],
                                start=False,
                                stop=True,
                                perf_mode=mybir.MatmulPerfMode.DoubleRow,
                            )
                        nc.tensor.matmul(
                            gl[:, hs], ylhs[:, isl], yrhs[:, js],
                            start=True, stop=True, tile_position=(yb, 0),
                        )
                    if mode == "peonly":
                        continue
                    # bufs=10: each segment's K/L tiles stay alive through
                    # the NEXT segment, so the deferred column-sum matmuls
                    # never make the in-order PE queue wait on ACT.
                    ksb = work.tile([128, 2 * B], bf16, tag="ksb", bufs=10)
                    lsb = work.tile([128, 2 * B], bf16, tag="lsb", bufs=10)
                    psb = None
                    if mode != "nodve":
                        psb = work.tile([128, 2 * B], bf16, tag="psb", bufs=6)
                    # Diagonal blocks are symmetric: row sums == colsums, so
                    # the ACT accumulator is skipped there.
                    nc.scalar.activation(
                        out=ksb[:, :jw],
                        in_=gk[:, :jw],
                        func=exp,
                        bias=bx[:, col : col + 1],
                        scale=2.0,
                        accum_out=None if diag else accK[:, col : col + 1],
                    )
                    nc.scalar.activation(
                        out=lsb[:, :jw],
                        in_=gl[:, :jw],
                        func=exp,
                        bias=by[:, col : col + 1],
                        scale=2.0,
                        accum_out=None if diag else accL[:, col : col + 1],
                    )
                    if mode != "nodve":
                        nc.vector.scalar_tensor_tensor(
                            out=psb[:, :jw],
                            in0=ksb[:, :jw],
                            scalar=1.0,
                            in1=lsb[:, :jw],
                            op0=mult,
                            op1=mult,
                            accum_out=accP[:, col : col + 1],
                        )
                    kls.append((ksb, lsb))
                # Column sums are deferred by one segment (emitted from the
                # pending list below) so their ACT-dependent semaphore waits
                # never head-of-line-block the next segment's compute
                # matmuls in the in-order PE queue.
                if mode not in ("nocolsum", "peonly"):
                    pending.append((seg, nh))
                    if len(pending) > 1:
                        emit_colsums(*pending.pop(0))
            while pending:
                emit_colsums(*pending.pop(0))

            nc.sync.dma_start(out=accK_d, in_=accK)
            nc.sync.dma_start(out=accL_d, in_=accL)
            nc.sync.dma_start(out=accP_d, in_=accP)

        if mode == "noload":
            loads()
        if reps > 1:
            with tc.For_i(0, reps):
                body()
        else:
            body()

    nc.compile()
    return nc


def _split_hi_lo(a):
    """Split float64 vector into hi+lo bf16 pair summing to ~a."""
    h = a.astype(ml_dtypes.bfloat16)
    l = (a - h.astype(np.float64)).astype(ml_dtypes.bfloat16)
    return h, l


def _prepare_in_maps(x, y):
    # Off-diagonal slots: x rounded to fp8e4m3 (TRN variant) for the
    # DoubleRow Gram matmul. Diagonal slots: bf16 Gram (dxx), because the
    # HW fp8 accumulator's tiny deficit breaks the exact diagonal
    # cancellation. Squared norms are computed from the rounded values of
    # whichever dtype that slot's Gram uses.
    xb8 = x.astype(ml_dtypes.float8_e4m3)
    xb16 = x.astype(ml_dtypes.bfloat16)
    yb = y.astype(ml_dtypes.bfloat16)
    sqx8 = (xb8.astype(np.float64) ** 2).sum(axis=1)  # [M]
    sqx16 = (xb16.astype(np.float64) ** 2).sum(axis=1)
    sqy = (yb.astype(np.float64) ** 2).sum(axis=1)

    xsqh8, xsql8 = _split_hi_lo(-0.5 * sqx8)
    xsqh16, xsql16 = _split_hi_lo(-0.5 * sqx16)
    ysqh, ysql = _split_hi_lo(-0.5 * sqy)

    xtb8 = np.ascontiguousarray(xb8.T)  # [DX, M]
    xtb16 = np.ascontiguousarray(xb16.T)
    ytb = np.ascontiguousarray(yb.T)  # [DY, M]
    xsq2_8 = np.stack([xsqh8, xsql8], axis=0)  # [2, M]
    xsq2_16 = np.stack([xsqh16, xsql16], axis=0)
    ysq2 = np.stack([ysqh, ysql], axis=0)
    ones_row = np.ones((2, M), dtype=ml_dtypes.bfloat16)
    ylhs_full = np.concatenate([ytb, ones_row], axis=0)  # [18, M]
    yrhs_full = np.concatenate([ytb, ysq2], axis=0)

    bslice = lambda a, blk: a[..., blk * B : (blk + 1) * B]

    in_maps = []
    for slots in _core_slots():
        is_diag = [I == J for I, J in slots]
        lhsx = np.concatenate([bslice(xtb8, I) for I, _ in slots], axis=1)
        rhsx = np.concatenate([bslice(xtb8, J) for _, J in slots], axis=1)
        # DoubleRow layout: [64, 2, W], (k2, j) <-> feature dim 2*k2+j
        lhsx = lhsx.reshape(DX // 2, 2, W)
        rhsx = rhsx.reshape(DX // 2, 2, W)
        dxx = np.concatenate(
            [bslice(xtb16, slots[s][0]) for s in range(2)], axis=1
        )
        xsq = np.concatenate(
            [
                bslice(xsq2_16 if dg else xsq2_8, J)
                for (_, J), dg in zip(slots, is_diag)
            ],
            axis=1,
        )
        ylhs = np.concatenate([bslice(ylhs_full, I) for I, _ in slots], axis=1)
        yrhs = np.concatenate([bslice(yrhs_full, J) for _, J in slots], axis=1)
        bxc = np.concatenate(
            [
                -(sqx16 if dg else sqx8)[I * B : (I + 1) * B].reshape(TPB, 128).T
                for (I, _), dg in zip(slots, is_diag)
            ],
            axis=1,
        ).astype(np.float32)
        byc = np.concatenate(
            [-sqy[I * B : (I + 1) * B].reshape(TPB, 128).T for I, _ in slots], axis=1
        ).astype(np.float32)
        in_maps.append(
            {
                "lhsx": np.ascontiguousarray(lhsx),
                "rhsx": np.ascontiguousarray(rhsx),
                "dxx": np.ascontiguousarray(dxx),
                "xsq": np.ascontiguousarray(xsq),
                "ylhs": np.ascontiguousarray(ylhs),
                "yrhs": np.ascontiguousarray(yrhs),
                "bxy": np.ascontiguousarray(np.concatenate([bxc, byc], axis=1)),
            }
        )
    return in_maps


def _combine(results):
    """Host-side reduction of per-core partial sums -> hsic scalar."""
    m = float(M)
    kv = np.zeros(M, dtype=np.float64)
    lv = np.zeros(M, dtype=np.float64)
    s_lk = 0.0
    for slots, res in zip(_core_slots(), results):
        aK = res["accK"].astype(np.float64)  # [128, NACC]
        aL = res["accL"].astype(np.float64)
        aP = res["accP"].astype(np.float64)
        cKL = res["colKL"].astype(np.float64)  # [4*NSEG, B]
        for seg, (s0, nh, diag) in enumerate(SEGMENTS):
            I = slots[s0][0]
            p_blk = aP[:, s0 * TPB : (s0 + 1) * TPB].sum()
            s_lk += p_blk if diag else 2.0 * p_blk
            if not diag:
                # row-side sums (cover all nh half-blocks' columns)
                for t in range(TPB):
                    rows = slice(I * B + t * 128, I * B + (t + 1) * 128)
                    kv[rows] += aK[:, s0 * TPB + t]
                    lv[rows] += aL[:, s0 * TPB + t]
            # col-side sums (for diag blocks these are also the row sums)
            for h in range(nh):
                J = slots[s0 + h][1]
                jrows = slice(J * B, (J + 1) * B)
                kv[jrows] += cKL[4 * seg + 2 * h]
                lv[jrows] += cKL[4 * seg + 2 * h + 1]
    sK = kv.sum()
    sL = lv.sum()
    hsic = (s_lk - (2.0 / m) * np.dot(kv, lv) + sK * sL / (m * m)) / (m - 1.0) ** 2
    return np.float32(hsic)


def get_program(reps=1, mode="full"):
    key = ("nc", reps, mode)
    if key not in _CACHE:
        _CACHE[key] = _build_program(reps, mode)
    return _CACHE[key]


def run_on_cores(in_maps):
    from concourse.bass_utils import run_bass_kernel_spmd

    nc = get_program()
    res = run_bass_kernel_spmd(nc, in_maps, core_ids=list(range(NCORES)))
    return res.results


def kernel(x, y):
    x = np.asarray(x)
    y = np.asarray(y)
    assert x.shape == (M, DX) and y.shape == (M, DY), (x.shape, y.shape)
    in_maps = _prepare_in_maps(x, y)
    results = run_on_cores(in_maps)
    return _combine(results)


# revision 78
# speedup vs baseline: 1.0820x; 1.0270x over previous
"""HSIC loss kernel for Trainium2 (8 NeuronCores, Bass/Tile).

hsic = sum(L * HKH) / (m-1)^2
     = (S_LK - (2/m) kv.lv + sK*sL/m^2) / (m-1)^2
where K = exp(-dx), L = exp(-dy) (Gaussian kernels, sigma=1),
kv/lv = row sums of K/L, sK/sL = total sums, S_LK = sum(K*L).

Sharding (symmetry-aware): the m x m pair space is tiled into 512x512
blocks; only the upper triangle (136 blocks = 8 cores x 17 slots, each core
getting 2 diagonal + 15 off-diagonal blocks) is computed. An off-diagonal
block (I,J) contributes its row sums to rows I (free-axis accumulation on
the Scalar engine) and, via K's symmetry, its column sums to rows J
(selector-column matmul on the Tensor engine). Its K*L sum counts twice.
Host combines the tiny per-core partial sums in float64.

On-chip per 128x512 tile:
  PE  : G_K = x_i.x_j - sqx_j/2   (bf16 matmul + rank-2 hi/lo -sqx/2 rows)
        G_L = y_i.y_j - sqy_j/2   (single K=18 augmented matmul)
  ACT : K = exp(2*G_K + bias_i),  bias_i = -sqx_i  (per-partition, fp32),
        accum_out giving the row sums for free. Same for L.
  DVE : P = K*L with accum_out giving sum(K*L) partials.
  PE  : column sums of K and L accumulate into one [4, 512] PSUM tile
        (partition r selected by a one-hot-column stationary), drained by
        a single DVE copy + DMA per segment.

All matmuls are bf16; exactness of the Gram diagonal (the only entries that
matter at fp32 scale) is preserved by computing the squared norms on host
from the *bf16-rounded* inputs and carrying -sq/2 as a hi/lo bf16 pair.

_build_program(reps) wraps the whole body in a hardware For_i loop; the
test harness times reps=R vs reps=1 to get the marginal per-iteration
device time, independent of host dispatch overhead.
"""

import numpy as np
import ml_dtypes

M = 8192
DX = 128
DY = 16
NCORES = 8
B = 512                  # block edge
NBLK = M // B            # 16 blocks per edge
NSLOT = 17               # blocks per core: 136 = 8*17
TPB = B // 128           # i-tiles per block = 4
NACC = NSLOT * TPB       # accumulator columns = 68
W = NSLOT * B            # gathered free width = 8704

# Segments: (base slot, n half-blocks, is_diag)
SEGMENTS = (
    [(0, 1, True), (1, 1, True)]
    + [(2 + 2 * p, 2, False) for p in range(7)]
    + [(16, 1, False)]
)
NSEG = len(SEGMENTS)     # 10

_CACHE = {}

# Concurrency switches: place the aug / y matmuls on their own PE row
# groups (rows 64-65 / 96-113) so they overlap the fp8-DoubleRow x-matmul
# (rows 0-63) on hardware. Serial placement (base 0) forces FIFO order.
AUG_SERIAL = False
Y_SERIAL = False
# gl double-buffered, gk single: measured faster once column sums are
# deferred (the PSUM budget only fits one double-buffered Gram tile).
GK_BUFS = 1


def _core_slots():
    """Per-core block lists: [(I,J), ...] len 17.

    Positional layout: slots 0-1 diagonal singles; slots (2,3),(4,5)...(14,15)
    are same-I pairs (processed as one 1024-wide tile); slot 16 a single.
    120 off-diag blocks = 56 same-I pairs + 8 singles = 8 cores x (7 pairs + 1).
    """
    diag = [(d, d) for d in range(NBLK)]
    pairs, singles = [], []
    for i in range(NBLK):
        row = [(i, j) for j in range(i + 1, NBLK)]
        while len(row) >= 2:
            pairs.append((row.pop(0), row.pop(0)))
        if row:
            singles.append(row[0])
    assert len(pairs) == 7 * NCORES and len(singles) == NCORES
    slots = []
    for c in range(NCORES):
        sl = [diag[2 * c], diag[2 * c + 1]]
        for a, b in pairs[c::NCORES]:
            sl += [a, b]
        sl.append(singles[c])
        slots.append(sl)
    return slots


def _build_program(reps=1, mode="full"):
    """Build + compile the SPMD Bass program (identical for all cores).

    mode: "full" = normal; "noload" = input DMAs hoisted out of the reps
    loop (probe: is the loop DMA-bound?); "dmaonly" = loads but no compute
    (probe: pure DMA+sync cost). Probe modes are for timing bisection only.
    """
    from contextlib import ExitStack

    import concourse.bacc as bacc
    import concourse.tile as tile
    from concourse import mybir

    nc = bacc.Bacc(
        "TRN2",
        target_bir_lowering=False,
        debug=False,
        num_devices=NCORES,
    )
    bf16 = mybir.dt.bfloat16
    f32 = mybir.dt.float32
    f8 = mybir.dt.float8e4

    # Per-core DRAM inputs (host gathers the per-slot I/J column blocks).
    # The narrow tensors (xsq/ylhs/yrhs, few partitions each) load per-slot
    # from the Pool SWDGE queue whose triggers cost ~25ns, keeping the
    # in-order SP queue (500ns/trigger) free for the wide lhsx/rhsx stream
    # and away from the per-segment colKL drains.
    # x rides in fp8 WITHOUT DoubleRow: same 1-elem/cell/cycle matmul rate
    # as bf16, but the full-128-row weight qualifies for the compiler's
    # fast-weight-load path (DoubleRow would disable FWL and pay +72% on
    # the serialized LDWEIGHTS stream). The fp8 rounding is absorbed
    # exactly by computing the squared norms from the rounded values.
    lhsx_d = nc.dram_tensor("lhsx", [DX, W], f8, kind="ExternalInput").ap()
    rhsx_d = nc.dram_tensor("rhsx", [DX, W], f8, kind="ExternalInput").ap()
    # The two diagonal slots use a bf16 Gram instead: the fp8 DoubleRow
    # accumulator on HW carries a ~1e-4-relative deficit, visible only where
    # the Gram diagonal must cancel the squared norms exactly. dxx holds the
    # diag slots' columns (I == J, so one tensor serves both operands).
    dxx_d = nc.dram_tensor("dxx", [DX, 2 * B], bf16, kind="ExternalInput").ap()
    xsq_d = nc.dram_tensor("xsq", [2, W], bf16, kind="ExternalInput").ap()
    ylhs_d = nc.dram_tensor("ylhs", [DY + 2, W], bf16, kind="ExternalInput").ap()
    yrhs_d = nc.dram_tensor("yrhs", [DY + 2, W], bf16, kind="ExternalInput").ap()
    bxy_d = nc.dram_tensor("bxy", [128, 2 * NACC], f32, kind="ExternalInput").ap()

    accK_d = nc.dram_tensor("accK", [128, NACC], f32, kind="ExternalOutput").ap()
    accL_d = nc.dram_tensor("accL", [128, NACC], f32, kind="ExternalOutput").ap()
    accP_d = nc.dram_tensor("accP", [128, NACC], f32, kind="ExternalOutput").ap()
    # Column sums, streamed per segment: rows 4s..4s+2*nh hold
    # [K h0, L h0, K h1, L h1] for segment s.
    colKL_d = nc.dram_tensor("colKL", [4 * NSEG, B], f32, kind="ExternalOutput").ap()

    with tile.TileContext(nc) as tc, ExitStack() as ctx:
        singles = ctx.enter_context(tc.tile_pool(name="singles", bufs=1))
        work = ctx.enter_context(tc.tile_pool(name="work", bufs=4))
        psum = ctx.enter_context(tc.tile_pool(name="psum", bufs=2, space="PSUM"))

        # Operands of a tile_position=(r, 0) matmul must live at SBUF base
        # partition r (walrus: in_base_partition == tile_pos). The aug pair
        # sits at partitions 64-65, the y operands at 96-113; allocation
        # cost is unchanged (SBUF reserves the byte range, not partitions).
        ab = 0 if AUG_SERIAL else 64
        yb = 0 if Y_SERIAL else 96
        lhsx = singles.tile([DX, W], f8)
        rhsx = singles.tile([DX, W], f8)
        dxx = singles.tile([DX, 2 * B], bf16)
        xsq_t = singles.tile([ab + 2, W], bf16)
        xsq = xsq_t[ab:, :]
        ylhs_t = singles.tile([yb + DY + 2, W], bf16)
        ylhs = ylhs_t[yb:, :]
        yrhs_t = singles.tile([yb + DY + 2, W], bf16)
        yrhs = yrhs_t[yb:, :]
        bxy = singles.tile([128, 2 * NACC], f32)
        bx = bxy[:, :NACC]
        by = bxy[:, NACC:]
        ones2_t = singles.tile([ab + 2, 128], bf16)
        ones2 = ones2_t[ab:, :]
        ones128 = singles.tile([128, 1], bf16)
        accK = singles.tile([128, NACC], f32)
        accL = singles.tile([128, NACC], f32)
        accP = singles.tile([128, NACC], f32)

        nc.vector.memset(ones2, 1.0)
        nc.vector.memset(ones128, 1.0)

        exp = mybir.ActivationFunctionType.Exp
        mult = mybir.AluOpType.mult

        # Dummy activation at t~0 pulls the exp table load off the
        # startup-critical path (it would otherwise run after the first
        # tile's data wait).
        warm = singles.tile([1, 8], f32)
        nc.vector.memset(warm, 0.0)
        nc.scalar.activation(
            out=warm, in_=warm, func=exp, bias=0.0, scale=1.0
        )

        def loads():
            # Slot 0 loads first (compute-critical), then the bulk prefetch,
            # all on SP's HWDGE queue. SP only carries input prefetch, which
            # is self-paced; the per-segment output drains go via Pool.
            j0 = slice(0, B)
            nc.sync.dma_start(out=bxy, in_=bxy_d)
            nc.sync.dma_start(out=xsq[:, j0], in_=xsq_d[:, j0])
            nc.sync.dma_start(out=dxx[:, :B], in_=dxx_d[:, :B])
            nc.sync.dma_start(out=ylhs[:, j0], in_=ylhs_d[:, j0])
            nc.sync.dma_start(out=yrhs[:, j0], in_=yrhs_d[:, j0])
            nc.sync.dma_start(out=dxx[:, B:], in_=dxx_d[:, B:])
            nc.sync.dma_start(out=rhsx[:, j0], in_=rhsx_d[:, j0])
            nc.sync.dma_start(out=lhsx[:, j0], in_=lhsx_d[:, j0])
            for s in range(1, NSLOT):
                js = slice(s * B, (s + 1) * B)
                nc.sync.dma_start(out=xsq[:, js], in_=xsq_d[:, js])
                nc.sync.dma_start(out=rhsx[:, js], in_=rhsx_d[:, js])
                nc.sync.dma_start(out=lhsx[:, js], in_=lhsx_d[:, js])
                nc.sync.dma_start(out=ylhs[:, js], in_=ylhs_d[:, js])
                nc.sync.dma_start(out=yrhs[:, js], in_=yrhs_d[:, js])

        def body():
            if mode == "empty":
                nc.gpsimd.memset(accK, 0.0)
                nc.gpsimd.memset(accL, 0.0)
                nc.gpsimd.memset(accP, 0.0)
                nc.sync.dma_start(out=accK_d, in_=accK)
                nc.sync.dma_start(out=accL_d, in_=accL)
                nc.sync.dma_start(out=accP_d, in_=accP)
                return
            if mode != "noload":
                loads()
            if mode == "dmaonly":
                return
            nc.gpsimd.memset(accK, 0.0)
            nc.gpsimd.memset(accL, 0.0)
            nc.gpsimd.memset(accP, 0.0)

            pending = []
            seg_tiles = {}

            def emit_colsums(seg, nh):
                # Column sums, packed into one PSUM bank at partitions
                # 0/32 (half 0) and 64/96 (half 1) via tile_position
                # column tiling, accumulated over the segment's 4 tiles.
                cb = psum.tile([128, B], f32, tag="cb", bufs=2)
                for t, (ksb_t, lsb_t) in enumerate(seg_tiles.pop(seg)):
                    for h in range(nh):
                        hs = slice(h * B, (h + 1) * B)
                        for q, src in enumerate((ksb_t, lsb_t)):
                            pk = 64 * h + 32 * q
                            nc.tensor.matmul(
                                cb[pk : pk + 1, :],
                                ones128,
                                src[:, hs],
                                start=(t == 0),
                                stop=(t == TPB - 1),
                                tile_position=(0, pk),
                            )
                # Drain rows 0/32/64/96. Engine APs must start at a
                # 32-aligned partition, so the SBUF staging tile keeps the
                # same row positions and each row DMAs separately. Early
                # segments drain via Pool's SWDGE queue (keeps SP's
                # in-order queue free for input prefetch); the tail
                # segments use SP's lower-latency HWDGE path.
                nrow = 2 * nh
                ckl = work.tile([97, B], f32, tag="ckl", bufs=2)
                for r in range(nrow):
                    nc.vector.tensor_copy(
                        out=ckl[32 * r : 32 * r + 1],
                        in_=cb[32 * r : 32 * r + 1, :],
                    )
                eng = nc.sync if seg >= NSEG - 2 else nc.gpsimd
                for r in range(nrow):
                    eng.dma_start(
                        out=colKL_d[4 * seg + r : 4 * seg + r + 1, :],
                        in_=ckl[32 * r : 32 * r + 1],
                    )

            for seg, (s0, nh, diag) in enumerate(SEGMENTS):
                jw = nh * B
                kls = seg_tiles.setdefault(seg, [])
                for t in range(TPB):
                    col = s0 * TPB + t
                    isl = slice(s0 * B + t * 128, s0 * B + (t + 1) * 128)
                    # With column sums deferred a full segment, one of gk/gl
                    # can double-buffer (PSUM fits 8 banks total with cb).
                    gk = psum.tile([128, 2 * B], f32, tag="gk", bufs=GK_BUFS)
                    gl = psum.tile([128, 2 * B], f32, tag="gl", bufs=3 - GK_BUFS)
                    for h in range(nh):
                        js = slice((s0 + h) * B, (s0 + h + 1) * B)
                        hs = slice(h * B, (h + 1) * B)
                        # The fp8 DoubleRow x-matmul occupies only array
                        # rows 0-63, so the aug (rows 64-65) and y (rows
                        # 96-113) matmuls run concurrently with it on HW.
                        # aug is issued first with start=True; its 2-deep
                        # pipeline drains each column ~60 cycles before the
                        # x-matmul's 64-deep one, so the clear always lands
                        # before the accumulate.
                        if mode == "nox":
                            nc.tensor.matmul(
                                gk[:, hs], ones2, xsq[:, js],
                                start=True, stop=True, tile_position=(ab, 0),
                            )
                        elif diag:
                            # bf16 Gram for the diagonal slots (exact
                            # cancellation of the squared norms).
                            nc.tensor.matmul(
                                gk[:, hs], ones2, xsq[:, js],
                                start=True, stop=False, tile_position=(ab, 0),
                            )
                            nc.tensor.matmul(
                                gk[:, hs],
                                dxx[:, s0 * B + t * 128 : s0 * B + (t + 1) * 128],
                                dxx[:, s0 * B : (s0 + 1) * B],
                                start=False,
                                stop=True,
                            )
                        else:
                            nc.tensor.matmul(
                                gk[:, hs], ones2, xsq[:, js],
                                start=True, stop=False, tile_position=(ab, 0),
                            )
                            nc.tensor.matmul(
                                gk[:, hs],
                                lhsx[:, isl],
                                rhsx[:, js],
                                start=False,
                                stop=True,
                            )
                        nc.tensor.matmul(
                            gl[:, hs], ylhs[:, isl], yrhs[:, js],
                            start=True, stop=True, tile_position=(yb, 0),
                        )
                    if mode == "peonly":
                        continue
                    # bufs=10: each segment's K/L tiles stay alive through
                    # the NEXT segment, so the deferred column-sum matmuls
                    # never make the in-order PE queue wait on ACT.
                    ksb = work.tile([128, 2 * B], bf16, tag="ksb", bufs=10)
                    lsb = work.tile([128, 2 * B], bf16, tag="lsb", bufs=10)
                    psb = None
                    if mode != "nodve":
                        psb = work.tile([128, 2 * B], bf16, tag="psb", bufs=6)
                    # Diagonal blocks are symmetric: row sums == colsums, so
                    # the ACT accumulator is skipped there.
                    nc.scalar.activation(
                        out=ksb[:, :jw],
                        in_=gk[:, :jw],
                        func=exp,
                        bias=bx[:, col : col + 1],
                        scale=2.0,
                        accum_out=None if diag else accK[:, col : col + 1],
                    )
                    nc.scalar.activation(
                        out=lsb[:, :jw],
                        in_=gl[:, :jw],
                        func=exp,
                        bias=by[:, col : col + 1],
                        scale=2.0,
                        accum_out=None if diag else accL[:, col : col + 1],
                    )
                    if mode != "nodve":
                        nc.vector.scalar_tensor_tensor(
                            out=psb[:, :jw],
                            in0=ksb[:, :jw],
                            scalar=1.0,
                            in1=lsb[:, :jw],
                            op0=mult,
                            op1=mult,
                            accum_out=accP[:, col : col + 1],
                        )
                    kls.append((ksb, lsb))
                # Column sums are deferred by one segment (emitted from the
                # pending list below) so their ACT-dependent semaphore waits
                # never head-of-line-block the next segment's compute
                # matmuls in the in-order PE queue.
                if mode not in ("nocolsum", "peonly"):
                    pending.append((seg, nh))
                    if len(pending) > 1:
                        emit_colsums(*pending.pop(0))
            while pending:
                emit_colsums(*pending.pop(0))

            nc.sync.dma_start(out=accK_d, in_=accK)
            nc.sync.dma_start(out=accL_d, in_=accL)
            nc.sync.dma_start(out=accP_d, in_=accP)

        if mode == "noload":
            loads()
        if reps > 1:
            with tc.For_i(0, reps):
                body()
        else:
            body()

    nc.compile()
    return nc


def _split_hi_lo(a):
    """Split float64 vector into hi+lo bf16 pair summing to ~a."""
    h = a.astype(ml_dtypes.bfloat16)
    l = (a - h.astype(np.float64)).astype(ml_dtypes.bfloat16)
    return h, l


def _prepare_in_maps(x, y):
    # Off-diagonal slots: x rounded to fp8e4m3 (TRN variant) for the
    # DoubleRow Gram matmul. Diagonal slots: bf16 Gram (dxx), because the
    # HW fp8 accumulator's tiny deficit breaks the exact diagonal
    # cancellation. Squared norms are computed from the rounded values of
    # whichever dtype that slot's Gram uses.
    xb8 = x.astype(ml_dtypes.float8_e4m3)
    xb16 = x.astype(ml_dtypes.bfloat16)
    yb = y.astype(ml_dtypes.bfloat16)
    sqx8 = (xb8.astype(np.float64) ** 2).sum(axis=1)  # [M]
    sqx16 = (xb16.astype(np.float64) ** 2).sum(axis=1)
    sqy = (yb.astype(np.float64) ** 2).sum(axis=1)

    xsqh8, xsql8 = _split_hi_lo(-0.5 * sqx8)
    xsqh16, xsql16 = _split_hi_lo(-0.5 * sqx16)
    ysqh, ysql = _split_hi_lo(-0.5 * sqy)

    xtb8 = np.ascontiguousarray(xb8.T)  # [DX, M]
    xtb16 = np.ascontiguousarray(xb16.T)
    ytb = np.ascontiguousarray(yb.T)  # [DY, M]
    xsq2_8 = np.stack([xsqh8, xsql8], axis=0)  # [2, M]
    xsq2_16 = np.stack([xsqh16, xsql16], axis=0)
    ysq2 = np.stack([ysqh, ysql], axis=0)
    ones_row = np.ones((2, M), dtype=ml_dtypes.bfloat16)
    ylhs_full = np.concatenate([ytb, ones_row], axis=0)  # [18, M]
    yrhs_full = np.concatenate([ytb, ysq2], axis=0)

    bslice = lambda a, blk: a[..., blk * B : (blk + 1) * B]

    in_maps = []
    for slots in _core_slots():
        is_diag = [I == J for I, J in slots]
        lhsx = np.concatenate([bslice(xtb8, I) for I, _ in slots], axis=1)
        rhsx = np.concatenate([bslice(xtb8, J) for _, J in slots], axis=1)
        dxx = np.concatenate(
            [bslice(xtb16, slots[s][0]) for s in range(2)], axis=1
        )
        xsq = np.concatenate(
            [
                bslice(xsq2_16 if dg else xsq2_8, J)
                for (_, J), dg in zip(slots, is_diag)
            ],
            axis=1,
        )
        ylhs = np.concatenate([bslice(ylhs_full, I) for I, _ in slots], axis=1)
        yrhs = np.concatenate([bslice(yrhs_full, J) for _, J in slots], axis=1)
        bxc = np.concatenate(
            [
                -(sqx16 if dg else sqx8)[I * B : (I + 1) * B].reshape(TPB, 128).T
                for (I, _), dg in zip(slots, is_diag)
            ],
            axis=1,
        ).astype(np.float32)
        byc = np.concatenate(
            [-sqy[I * B : (I + 1) * B].reshape(TPB, 128).T for I, _ in slots], axis=1
        ).astype(np.float32)
        in_maps.append(
            {
                "lhsx": np.ascontiguousarray(lhsx),
                "rhsx": np.ascontiguousarray(rhsx),
                "dxx": np.ascontiguousarray(dxx),
                "xsq": np.ascontiguousarray(xsq),
                "ylhs": np.ascontiguousarray(ylhs),
                "yrhs": np.ascontiguousarray(yrhs),
                "bxy": np.ascontiguousarray(np.concatenate([bxc, byc], axis=1)),
            }
        )
    return in_maps


def _combine(results):
    """Host-side reduction of per-core partial sums -> hsic scalar."""
    m = float(M)
    kv = np.zeros(M, dtype=np.float64)
    lv = np.zeros(M, dtype=np.float64)
    s_lk = 0.0
    for slots, res in zip(_core_slots(), results):
        aK = res["accK"].astype(np.float64)  # [128, NACC]
        aL = res["accL"].astype(np.float64)
        aP = res["accP"].astype(np.float64)
        cKL = res["colKL"].astype(np.float64)  # [4*NSEG, B]
        for seg, (s0, nh, diag) in enumerate(SEGMENTS):
            I = slots[s0][0]
            p_blk = aP[:, s0 * TPB : (s0 + 1) * TPB].sum()
            s_lk += p_blk if diag else 2.0 * p_blk
            if not diag:
                # row-side sums (cover all nh half-blocks' columns)
                for t in range(TPB):
                    rows = slice(I * B + t * 128, I * B + (t + 1) * 128)
                    kv[rows] += aK[:, s0 * TPB + t]
                    lv[rows] += aL[:, s0 * TPB + t]
            # col-side sums (for diag blocks these are also the row sums)
            for h in range(nh):
                J = slots[s0 + h][1]
                jrows = slice(J * B, (J + 1) * B)
                kv[jrows] += cKL[4 * seg + 2 * h]
                lv[jrows] += cKL[4 * seg + 2 * h + 1]
    sK = kv.sum()
    sL = lv.sum()
    hsic = (s_lk - (2.0 / m) * np.dot(kv, lv) + sK * sL / (m * m)) / (m - 1.0) ** 2
    return np.float32(hsic)


def get_program(reps=1, mode="full"):
    key = ("nc", reps, mode)
    if key not in _CACHE:
        _CACHE[key] = _build_program(reps, mode)
    return _CACHE[key]


def run_on_cores(in_maps):
    from concourse.bass_utils import run_bass_kernel_spmd

    nc = get_program()
    res = run_bass_kernel_spmd(nc, in_maps, core_ids=list(range(NCORES)))
    return res.results


def kernel(x, y):
    x = np.asarray(x)
    y = np.asarray(y)
    assert x.shape == (M, DX) and y.shape == (M, DY), (x.shape, y.shape)
    in_maps = _prepare_in_maps(x, y)
    results = run_on_cores(in_maps)
    return _combine(results)
